# revision 1
# baseline (speedup 1.0000x reference)
"""Trainium2 Bass kernel for nn_MHSA_40346922778634.

Math (per batch b, head h; the reference computes-then-drops the register
group, so reg_qk/reg_v are dead inputs):
  X = x[b] as [C=512, N=1024]
  Q = Wq X + bq ; K = Wk X + bk ; V = Wv X + bv       (per head: [64, N])
  P_h = (rel_h + rel_w) reshaped [head, 64, N]
  E[i,j] = Q_h[:,i].K_h[:,j] + P_h[:,i].Q_h[:,j]      ([N, N])
  attn = softmax(E, axis=-1)
  Out_h = V_h @ attn^T ; out[b, h*64:(h+1)*64] = Out_h + X[h*64:(h+1)*64]

Kernel strategy (8 cores, data-parallel over batch, 2 batches/core):
  - fp16 operands for projection + energy matmuls (fp32 runs at 1/4 PE rate;
    fp16 streams at bf16 rate with 8x the mantissa of bf16 -> ~0.18% error).
  - E^T = Z^T U with U = [Q_h; P_h], Z = [K_h; Q_h] stacked to K=128
    (one matmul computes both energy terms).
  - exp without max-subtraction (logits bounded ~ +-35, safe in fp32 PSUM),
    T^T = exp(E^T) stored bf16 (range needs bf16, fp16 would overflow).
  - denominator via ones-augmented V^T (padded layout, 65 cols per head with
    the 65th = 1.0), AV matmuls bf16.
  - normalize: reciprocal (fp32) split into bf16 hi+lo, broadcast via two
    rank-1 matmuls, DVE multiply + residual add, store fp32.
"""

import sys

import numpy as np

try:
    import concourse.bass as bass  # noqa: F401
except Exception:  # pragma: no cover
    sys.path.insert(0, "/opt/trn_rl_repo")

import concourse.bass as bass  # noqa: F401
import concourse.tile as tile
from concourse import bacc, mybir
from concourse.bass_utils import run_bass_kernel_spmd

F32 = mybir.dt.float32
F16 = mybir.dt.float16
BF16 = mybir.dt.bfloat16
EXP = mybir.ActivationFunctionType.Exp

N_CORES = 8
B, C, WD, HD = 16, 512, 32, 32
HEAD, D, N = 8, 64, 1024
BPC = B // N_CORES  # batches per core


def build_bass():
    nc = bacc.Bacc("TRN2")

    xs_d = nc.dram_tensor("xs", [BPC, C, N], F32, kind="ExternalInput")
    xh_d = nc.dram_tensor("xh", [BPC, C, N], F16, kind="ExternalInput")
    wqt_d = nc.dram_tensor("wqt", [4, 128, 512], F16, kind="ExternalInput")
    wkt_d = nc.dram_tensor("wkt", [4, 128, 512], F16, kind="ExternalInput")
    wvpt_d = nc.dram_tensor("wvpt", [4, 128, 520], F16, kind="ExternalInput")
    bq_d = nc.dram_tensor("bq", [4, 128, 1], F32, kind="ExternalInput")
    bk_d = nc.dram_tensor("bk", [4, 128, 1], F32, kind="ExternalInput")
    bvp_d = nc.dram_tensor("bvp", [1, 520], F16, kind="ExternalInput")
    pos_d = nc.dram_tensor("pos", [HEAD, D, N], F16, kind="ExternalInput")
    out_d = nc.dram_tensor("out", [BPC, C, N], F32, kind="ExternalOutput")

    with tile.TileContext(nc) as tc:
        with (
            tc.tile_pool(name="consts", bufs=1) as cpool,
            tc.tile_pool(name="work", bufs=2) as wpool,
            tc.tile_pool(name="psume", bufs=2, space="PSUM") as pse,
            tc.tile_pool(name="psumo", bufs=2, space="PSUM") as pso,
        ):
            # ---- batch-0 X first (unblocks first projection ASAP) ----
            def prep_x(b):
                x_sb = wpool.tile([128, 4, N], F16, name=f"x_{b}", tag="x")
                for kc in range(4):
                    nc.sync.dma_start(x_sb[:, kc, :], xh_d[b, kc * 128:(kc + 1) * 128, :])
                return x_sb

            ctx = {0: {}}
            ctx[0]["x"] = prep_x(0)

            # ---- constants ----
            wqt_sb = cpool.tile([128, 4, 512], F16, name="wqt_sb")
            wkt_sb = cpool.tile([128, 4, 512], F16, name="wkt_sb")
            wvpt_sb = cpool.tile([128, 4, 520], F16, name="wvpt_sb")
            for kc in range(4):
                nc.sync.dma_start(wqt_sb[:, kc, :], wqt_d[kc])
                nc.sync.dma_start(wkt_sb[:, kc, :], wkt_d[kc])
                nc.sync.dma_start(wvpt_sb[:, kc, :], wvpt_d[kc])
            bq_sb = cpool.tile([128, 4, 1], F32, name="bq_sb")
            bk_sb = cpool.tile([128, 4, 1], F32, name="bk_sb")
            for mc in range(4):
                nc.sync.dma_start(bq_sb[:, mc, :], bq_d[mc])
                nc.sync.dma_start(bk_sb[:, mc, :], bk_d[mc])
            bvp_sb = cpool.tile([1, 520], F16, name="bvp_sb")
            nc.sync.dma_start(bvp_sb[:], bvp_d[:])
            ones16 = cpool.tile([128, 128], F16, name="ones16")
            nc.vector.memset(ones16[:], 1.0)
            onesbf = cpool.tile([128, 128], BF16, name="onesbf")
            nc.vector.memset(onesbf[:], 1.0)
            zbias = cpool.tile([128, 1], F32, name="zbias")
            nc.vector.memset(zbias[:], 0.0)

            def emit_qkproj(b):
                # Q/K projections -> qall/kall (fp16, bias added); interleaved
                # per output chunk so head 0's Q and K drain early.
                cx = ctx[b]
                qall = wpool.tile([128, 4, N], F16, name=f"qall_{b}", tag="qall", bufs=2)
                kall = wpool.tile([128, 4, N], F16, name=f"kall_{b}", tag="kall", bufs=2)
                for mc in range(4):
                    for (wt, bt, dst, t0) in (
                        (wqt_sb, bq_sb, qall, "q"),
                        (wkt_sb, bk_sb, kall, "k"),
                    ):
                        for nh in range(2):
                            ps = pso.tile([128, 512], F32, name=f"ps_p{t0}{b}{mc}{nh}", tag="pso")
                            for kc in range(4):
                                nc.tensor.matmul(
                                    ps[:],
                                    wt[:, kc, mc * 128:(mc + 1) * 128],
                                    cx["x"][:, kc, nh * 512:(nh + 1) * 512],
                                    start=(kc == 0),
                                    stop=(kc == 3),
                                )
                            nc.vector.tensor_scalar_add(
                                dst[:, mc, nh * 512:(nh + 1) * 512], ps[:], bt[:, mc, :]
                            )
                cx["qall"], cx["kall"] = qall, kall

            def emit_vproj(b, c0, c1):
                # V^T padded projection (bf16 out), ones-column included via
                # the padded bias row. Chunk range [c0, c1) so the PE burst
                # can be spread between heads instead of starving ACT.
                cx = ctx[b]
                if "vpt" not in cx:
                    cx["vpt"] = wpool.tile([128, 8, 520], BF16, name=f"vpt_{b}", tag="vpt")
                vpt = cx["vpt"]
                for nc8 in range(c0, c1):
                    ps = pso.tile([128, 520], F32, name=f"ps_v{b}{nc8}", tag="pso")
                    for (lo, hi) in ((0, 512), (512, 520)):
                        for kc in range(4):
                            nc.tensor.matmul(
                                ps[:, lo:hi],
                                cx["x"][:, kc, nc8 * 128:(nc8 + 1) * 128],
                                wvpt_sb[:, kc, lo:hi],
                                start=(kc == 0),
                                stop=False,
                            )
                        nc.tensor.matmul(
                            ps[:, lo:hi],
                            ones16[0:1, 0:128],
                            bvp_sb[:, lo:hi],
                            start=False,
                            stop=True,
                        )
                    nc.vector.tensor_copy(vpt[:, nc8, :], ps[:])

            def emit_energy(b, h):
                # assembly of U = [Q_h; P_h], Z = [K_h; Q_h], then E^T + exp.
                # DVE cannot shift partitions: misaligned pieces go via DMA,
                # aligned pieces via the otherwise-idle GpSimd engine.
                cx = ctx[b]
                qall, kall = cx["qall"], cx["kall"]
                mc = h // 2
                u_h = wpool.tile([128, N], F16, name=f"u_{b}_{h}", tag="u", bufs=4)
                z_h = wpool.tile([128, N], F16, name=f"z_{b}_{h}", tag="z", bufs=4)
                nc.sync.dma_start(u_h[64:128, :], pos_d[h])
                if h % 2 == 0:
                    nc.gpsimd.tensor_copy(u_h[0:64, :], qall[0:64, mc, :])
                    nc.gpsimd.tensor_copy(z_h[0:64, :], kall[0:64, mc, :])
                    nc.sync.dma_start(z_h[64:128, :], qall[0:64, mc, :])
                else:
                    nc.sync.dma_start(u_h[0:64, :], qall[64:128, mc, :])
                    nc.sync.dma_start(z_h[0:64, :], kall[64:128, mc, :])
                    nc.gpsimd.tensor_copy(z_h[64:128, :], qall[64:128, mc, :])
                tts = []
                for j in range(8):
                    eps = pse.tile([128, N], F32, name=f"ps_e{b}{h}{j}", tag="pse")
                    for ih in range(2):
                        nc.tensor.matmul(
                            eps[:, ih * 512:(ih + 1) * 512],
                            z_h[:, j * 128:(j + 1) * 128],
                            u_h[:, ih * 512:(ih + 1) * 512],
                            start=True,
                            stop=True,
                        )
                    tt = wpool.tile([128, N], BF16, name=f"tt_{b}_{h}_{j}", tag="tt", bufs=18)
                    nc.scalar.activation(tt[:], eps[:], EXP, bias=zbias[:])
                    tts.append(tt)
                return tts

            def emit_out(b, h, tts):
                # AV: O = V_aug @ T (rows 0..63 numerator, row 64 denominator),
                # then normalize via bf16 denominator broadcast (rank-1
                # matmul), full-lane reciprocal, multiply, residual-add on
                # GpSimd, store fp32.
                vpt = ctx[b]["vpt"]
                ops = pso.tile([65, N], F32, name=f"ps_o{b}{h}", tag="pso")
                for mh in range(2):
                    for j in range(8):
                        nc.tensor.matmul(
                            ops[:, mh * 512:(mh + 1) * 512],
                            vpt[:, j, h * 65:h * 65 + 65],
                            tts[j][:, mh * 512:(mh + 1) * 512],
                            start=(j == 0),
                            stop=(j == 7),
                        )
                dhi = wpool.tile([65, N], BF16, name=f"dhi_{b}_{h}", tag="dhi")
                nc.vector.tensor_copy(dhi[64:65, :], ops[64:65, :])
                rps = pso.tile([64, N], F32, name=f"ps_r{b}{h}", tag="pso")
                for mh in range(2):
                    nc.tensor.matmul(
                        rps[:, mh * 512:(mh + 1) * 512],
                        onesbf[64:65, 0:64],
                        dhi[64:65, mh * 512:(mh + 1) * 512],
                        start=True,
                        stop=True,
                    )
                rbinv = wpool.tile([64, N], F32, name=f"rbinv_{b}_{h}", tag="rbinv")
                nc.vector.reciprocal(rbinv[:], rps[:])
                osb = wpool.tile([64, N], F32, name=f"osb_{b}_{h}", tag="ostage", bufs=3)
                nc.vector.tensor_mul(osb[:], ops[0:64, :], rbinv[:])
                xres = wpool.tile([64, N], F32, name=f"xres_{b}_{h}", tag="xres")
                nc.sync.dma_start(xres[:], xs_d[b, h * 64:(h + 1) * 64, :])
                fin = wpool.tile([64, N], F32, name=f"fin_{b}_{h}", tag="ostage", bufs=3)
                nc.gpsimd.tensor_add(fin[:], osb[:], xres[:])
                nc.sync.dma_start(out_d[b, h * 64:(h + 1) * 64, :], fin[:])

            # ---- software pipeline over (batch, head): AV lags one head
            # behind E/exp; next batch's X load + projections are emitted
            # mid-way through the previous batch's heads ----
            emit_qkproj(0)
            pend = None
            for b in range(BPC):
                for h in range(8):
                    tts = emit_energy(b, h)
                    if b == 0 and h < 2:
                        # AV(0) is emitted at h==1, so all 8 chunks must be
                        # emitted across h==0 and h==1.
                        emit_vproj(0, 4 * h, 4 * h + 4)
                    if b + 1 < BPC:
                        if h == 4:
                            ctx[b + 1] = {"x": prep_x(b + 1)}
                            emit_qkproj(b + 1)
                        elif h in (5, 6, 7):
                            emit_vproj(b + 1, 3 * (h - 5), min(8, 3 * (h - 5) + 3))
                    if pend is not None:
                        emit_out(*pend)
                    pend = (b, h, tts)
            emit_out(*pend)

    nc.compile()
    return nc


def _prep_consts(Wq, bq, Wk, bk, Wv, bv, rel_h, rel_w):
    wqt = np.ascontiguousarray(Wq.T).reshape(4, 128, 512).astype(np.float16)
    wkt = np.ascontiguousarray(Wk.T).reshape(4, 128, 512).astype(np.float16)
    wvpt = np.zeros((512, 520), np.float32)
    bvp = np.zeros((1, 520), np.float32)
    for h in range(HEAD):
        wvpt[:, h * 65:h * 65 + 64] = Wv[h * 64:(h + 1) * 64, :].T
        bvp[0, h * 65:h * 65 + 64] = bv[h * 64:(h + 1) * 64]
        bvp[0, h * 65 + 64] = 1.0
    pos = (rel_h + rel_w).reshape(HEAD, D, N).astype(np.float16)
    return {
        "wqt": wqt,
        "wkt": wkt,
        "wvpt": wvpt.reshape(4, 128, 520).astype(np.float16),
        "bq": bq.reshape(4, 128, 1).astype(np.float32),
        "bk": bk.reshape(4, 128, 1).astype(np.float32),
        "bvp": bvp.astype(np.float16),
        "pos": pos,
    }


_CACHE = {}


def build_in_maps(x, Wq, bq, Wk, bk, Wv, bv, rel_h, rel_w):
    x = np.asarray(x, np.float32)
    consts = _prep_consts(
        *[np.asarray(a, np.float32) for a in (Wq, bq, Wk, bk, Wv, bv, rel_h, rel_w)]
    )
    xr = x.reshape(B, C, N)
    xh = xr.astype(np.float16)
    in_maps = []
    for c in range(N_CORES):
        m = dict(consts)
        m["xs"] = np.ascontiguousarray(xr[c * BPC:(c + 1) * BPC])
        m["xh"] = np.ascontiguousarray(xh[c * BPC:(c + 1) * BPC])
        in_maps.append(m)
    return in_maps


def kernel(x, Wq, bq, Wk, bk, Wv, bv, rel_h, rel_w, reg_qk, reg_v):
    # reg_qk / reg_v are computed-then-dropped by the reference -> unused.
    in_maps = build_in_maps(x, Wq, bq, Wk, bk, Wv, bv, rel_h, rel_w)

    if "nc" not in _CACHE:
        _CACHE["nc"] = build_bass()
    res = run_bass_kernel_spmd(_CACHE["nc"], in_maps, list(range(N_CORES)))
    outs = [np.asarray(r["out"]) for r in res.results]
    return np.concatenate(outs, axis=0).reshape(B, C, WD, HD)


if __name__ == "__main__":
    nc = build_bass()
    print("built ok")



# revision 12
# speedup vs baseline: 1.7133x; 1.7133x over previous
"""Trainium2 Bass kernel for nn_MHSA_40346922778634.

Math (per batch b, head h; the reference computes-then-drops the register
group, so reg_qk/reg_v are dead inputs):
  X = x[b] as [C=512, N=1024]
  Q = Wq X + bq ; K = Wk X + bk ; V = Wv X + bv       (per head: [64, N])
  P_h = (rel_h + rel_w) reshaped [head, 64, N]
  E[n,m] = Q_h[:,n].K_h[:,m] + P_h[:,n].Q_h[:,m]      ([N, N])
  attn = softmax(E, axis=-1)  (over m)
  out[b, h*64:(h+1)*64] = V_h @ attn^T + X[h*64:(h+1)*64]

Kernel strategy (8 cores, data-parallel over batch, 2 batches/core):
  - Z-projection with interleaved weights Wz = [Wk_h; Wq_h] per head chunk
    produces Z_h = [K_h; Q_h] stacked on 128 partitions directly (no
    partition-shift copies).  U_h = [Q_h; P_h]: pos rows preloaded once into
    partitions 64-127, Q rows copied per head with one SBUF->SBUF DMA.
  - E^T = Z_h^T U_h, one K=128 matmul pass per 128-row chunk (fp16).
  - exp without max-subtraction (logits bounded, fp32 PSUM), T = exp(E^T)
    stored bf16 (needs bf16 range).
  - AV with ones-augmented V^T (65 cols per head, 65th = 1.0 -> denominator
    in row 64), bf16.  AV of head h-1 interleaved with energy of head h at
    j-chunk granularity to keep PE dense.
  - Unnormalized O staged to SBUF bf16; per batch ONE [8, N]
    reciprocal_approx_fast (DVE reciprocal cost scales with free-dim length
    only, so batching heads on partitions is 8x cheaper than per-head
    [64, N]), hi+lo bf16 split, per-head rank-1 broadcast matmul, DVE
    multiply, GpSimd residual add (fp16 x), fp16 store.  The normalize tail
    of batch b overlaps batch b+1 compute.
"""

import sys

import numpy as np

try:
    import concourse.bass as bass  # noqa: F401
except Exception:  # pragma: no cover
    sys.path.insert(0, "/opt/trn_rl_repo")

import ml_dtypes
import concourse.bass as bass  # noqa: F401
import concourse.tile as tile
from concourse import bacc, mybir
from concourse.bass_utils import run_bass_kernel_spmd

F32 = mybir.dt.float32
F16 = mybir.dt.float16
BF16 = mybir.dt.bfloat16
EXP = mybir.ActivationFunctionType.Exp

N_CORES = 8
B, C, WD, HD = 16, 512, 32, 32
HEAD, D, N = 8, 64, 1024
BPC = B // N_CORES  # batches per core


def build_bass():
    nc = bacc.Bacc("TRN2")

    xh_d = nc.dram_tensor("xh", [BPC, C, N], F16, kind="ExternalInput")
    wzt_d = nc.dram_tensor("wzt", [4, 128, 1024], F16, kind="ExternalInput")
    bz_d = nc.dram_tensor("bz", [128, 8], F32, kind="ExternalInput")
    wvpt_d = nc.dram_tensor("wvpt", [4, 128, 520], F16, kind="ExternalInput")
    bvp_d = nc.dram_tensor("bvp", [1, 520], F16, kind="ExternalInput")
    pos_d = nc.dram_tensor("pos", [HEAD, D, N], F16, kind="ExternalInput")
    mask_d = nc.dram_tensor("mask", [8, 512], BF16, kind="ExternalInput")
    out_d = nc.dram_tensor("out", [BPC, C, N], F16, kind="ExternalOutput")

    with tile.TileContext(nc) as tc:
        with (
            tc.tile_pool(name="consts", bufs=1) as cpool,
            tc.tile_pool(name="work", bufs=2) as wpool,
            tc.tile_pool(name="psume", bufs=2, space="PSUM") as pse,
            tc.tile_pool(name="psumo", bufs=4, space="PSUM") as pso,
        ):
            # ---- batch-0 X first (unblocks first projection ASAP) ----
            def prep_x(b):
                x_sb = wpool.tile([128, 4, N], F16, name=f"x_{b}", tag="x")
                for kc in range(4):
                    nc.sync.dma_start(x_sb[:, kc, :], xh_d[b, kc * 128:(kc + 1) * 128, :])
                return x_sb

            def prep_xodd(b):
                # odd heads' residual rows live at partitions 64-127 of x;
                # engines need matching base partitions, so shift them to 0.
                cx = ctx[b]
                xodd = wpool.tile([64, 4, N], F16, name=f"xodd_{b}", tag="xodd")
                for kc in range(4):
                    nc.sync.dma_start(xodd[:, kc, :], cx["x"][64:128, kc, :])
                cx["xodd"] = xodd

            ctx = {0: {}}
            ctx[0]["x"] = prep_x(0)
            prep_xodd(0)

            # ---- constants ----
            wzt_sb = cpool.tile([128, 4, 1024], F16, name="wzt_sb")
            wvpt_sb = cpool.tile([128, 4, 520], F16, name="wvpt_sb")
            for kc in range(4):
                nc.sync.dma_start(wzt_sb[:, kc, :], wzt_d[kc])
                nc.sync.dma_start(wvpt_sb[:, kc, :], wvpt_d[kc])
            bz_sb = cpool.tile([128, 8], F32, name="bz_sb")
            nc.sync.dma_start(bz_sb[:], bz_d[:])
            bvp_sb = cpool.tile([1, 520], F16, name="bvp_sb")
            nc.sync.dma_start(bvp_sb[:], bvp_d[:])
            mask_sb = cpool.tile([8, 512], BF16, name="mask_sb")
            nc.sync.dma_start(mask_sb[:], mask_d[:])
            ones1 = cpool.tile([1, 128], F16, name="ones1")
            nc.vector.memset(ones1[:], 1.0)
            zbias = cpool.tile([128, 1], F32, name="zbias")
            nc.vector.memset(zbias[:], 0.0)
            # U tiles: partitions 64-127 = pos (loaded once, reused across
            # batches), partitions 0-63 = Q_h (DMA'd per batch per head).
            uall = wpool.tile([128, 8, N], F16, name="uall", tag="uall", bufs=1)
            for h in range(HEAD):
                nc.sync.dma_start(uall[64:128, h, :], pos_d[h])

            def emit_zproj_chunk(b, h):
                # Z_h = [K_h; Q_h] directly from interleaved weights; then
                # prefetch U_h's Q rows with one SBUF->SBUF DMA.
                cx = ctx[b]
                if "zall" not in cx:
                    cx["zall"] = wpool.tile(
                        [128, 8, N], F16, name=f"zall_{b}", tag="zall", bufs=2
                    )
                zall = cx["zall"]
                for nh in range(2):
                    ps = pso.tile([128, 512], F32, name=f"ps_z{b}{h}{nh}", tag="pso")
                    for kc in range(4):
                        nc.tensor.matmul(
                            ps[:],
                            wzt_sb[:, kc, h * 128:(h + 1) * 128],
                            cx["x"][:, kc, nh * 512:(nh + 1) * 512],
                            start=(kc == 0),
                            stop=(kc == 3),
                        )
                    nc.vector.tensor_scalar_add(
                        zall[:, h, nh * 512:(nh + 1) * 512], ps[:], bz_sb[:, h:h + 1]
                    )
                nc.sync.dma_start(uall[0:64, h, :], zall[64:128, h, :])

            def emit_vproj(b, c0, c1):
                # V^T padded projection (bf16 out), ones-column included via
                # the padded bias row; main/tail split keeps PSUM slots 1-bank.
                cx = ctx[b]
                if "vpt" not in cx:
                    cx["vpt"] = wpool.tile([128, 8, 520], BF16, name=f"vpt_{b}", tag="vpt")
                vpt = cx["vpt"]
                for nc8 in range(c0, c1):
                    for (lo, hi) in ((0, 512), (512, 520)):
                        ps = pso.tile(
                            [128, hi - lo], F32, name=f"ps_v{b}{nc8}{lo}", tag="pso"
                        )
                        for kc in range(4):
                            nc.tensor.matmul(
                                ps[:],
                                cx["x"][:, kc, nc8 * 128:(nc8 + 1) * 128],
                                wvpt_sb[:, kc, lo:hi],
                                start=(kc == 0),
                                stop=False,
                            )
                        nc.tensor.matmul(
                            ps[:],
                            ones1[0:1, :],
                            bvp_sb[:, lo:hi],
                            start=False,
                            stop=True,
                        )
                        nc.vector.tensor_copy(vpt[:, nc8, lo:hi], ps[:])

            def get_osb(b):
                cx = ctx[b]
                if "osb" not in cx:
                    cx["osb"] = wpool.tile(
                        [65, 8, N], BF16, name=f"osb_{b}", tag="osb", bufs=2
                    )
                    cx["denf"] = wpool.tile(
                        [8, N], F32, name=f"denf_{b}", tag="denf", bufs=1
                    )
                return cx["osb"], cx["denf"]

            def emit_av_chunk(st, j):
                bp, hp, ptts, ops_a, ops_b = st
                pvpt = ctx[bp]["vpt"]
                for mh, ops in ((0, ops_a), (1, ops_b)):
                    nc.tensor.matmul(
                        ops[:],
                        pvpt[:, j, hp * 65:hp * 65 + 65],
                        ptts[j][:, mh * 512:(mh + 1) * 512],
                        start=(j == 0),
                        stop=(j == 7),
                    )

            def emit_av_evac(st):
                bp, hp, ptts, ops_a, ops_b = st
                osb, denf = get_osb(bp)
                nc.vector.tensor_copy(osb[:, hp, 0:512], ops_a[:])
                nc.vector.tensor_copy(osb[:, hp, 512:1024], ops_b[:])
                # SWDGE DMA casts bf16 -> fp32 while gathering den rows
                nc.gpsimd.dma_start(denf[hp:hp + 1, :], osb[64:65, hp, :])

            def emit_head(b, h, pend):
                # energy+exp for (b, h); AV for pend interleaved per j-chunk.
                cx = ctx[b]
                zall = cx["zall"]
                st = None
                if pend is not None:
                    bp, hp, ptts = pend
                    ops_a = pso.tile([65, 512], F32, name=f"ps_oa{bp}{hp}", tag="pso")
                    ops_b = pso.tile([65, 512], F32, name=f"ps_ob{bp}{hp}", tag="pso")
                    st = (bp, hp, ptts, ops_a, ops_b)
                tts = []
                for j in range(8):
                    eps = pse.tile([128, N], F32, name=f"ps_e{b}{h}{j}", tag="pse")
                    for ih in range(2):
                        nc.tensor.matmul(
                            eps[:, ih * 512:(ih + 1) * 512],
                            zall[:, h, j * 128:(j + 1) * 128],
                            uall[:, h, ih * 512:(ih + 1) * 512],
                            start=True,
                            stop=True,
                        )
                    if st is not None:
                        emit_av_chunk(st, j)
                    tt = wpool.tile([128, N], BF16, name=f"tt_{b}_{h}_{j}", tag="tt", bufs=18)
                    nc.scalar.activation(tt[:], eps[:], EXP, bias=zbias[:])
                    tts.append(tt)
                if st is not None:
                    emit_av_evac(st)
                return (b, h, tts)

            def emit_norm(b):
                # batch tail: one fast reciprocal for all 8 heads, hi+lo bf16
                # split, per-head rank-1 broadcast + normalize + residual.
                cx = ctx[b]
                osb, denf = cx["osb"], cx["denf"]
                rinv = wpool.tile([8, N], F32, name=f"rinv_{b}", tag="rinv", bufs=1)
                nc.vector.reciprocal_approx_fast(rinv[:], denf[:])
                hi = wpool.tile([8, N], BF16, name=f"hi_{b}", tag="hi", bufs=1)
                lo = wpool.tile([8, N], BF16, name=f"lo_{b}", tag="lo", bufs=1)
                nc.vector.tensor_copy(hi[:], rinv[:])
                nc.vector.tensor_sub(lo[:], rinv[:], hi[:])
                for h in range(HEAD):
                    nmul = wpool.tile([64, N], F16, name=f"nm_{b}_{h}", tag="nm", bufs=3)
                    for mh in range(2):
                        rb = pso.tile([64, 512], F32, name=f"ps_r{b}{h}{mh}", tag="pso")
                        nc.tensor.matmul(
                            rb[:],
                            mask_sb[:, h * 64:(h + 1) * 64],
                            hi[:, mh * 512:(mh + 1) * 512],
                            start=True,
                            stop=False,
                        )
                        nc.tensor.matmul(
                            rb[:],
                            mask_sb[:, h * 64:(h + 1) * 64],
                            lo[:, mh * 512:(mh + 1) * 512],
                            start=False,
                            stop=True,
                        )
                        nc.vector.tensor_mul(
                            nmul[:, mh * 512:(mh + 1) * 512],
                            osb[0:64, h, mh * 512:(mh + 1) * 512],
                            rb[:],
                        )
                    fin = wpool.tile([64, N], F16, name=f"fin_{b}_{h}", tag="fin", bufs=3)
                    if h % 2 == 0:
                        xres = cx["x"][0:64, h // 2, :]
                    else:
                        xres = cx["xodd"][:, h // 2, :]
                    nc.gpsimd.tensor_add(fin[:], nmul[:], xres)
                    nc.sync.dma_start(out_d[b, h * 64:(h + 1) * 64, :], fin[:])

            # ---- prologue: batch 0 projections, head 0 early ----
            emit_zproj_chunk(0, 0)
            pend = emit_head(0, 0, None)
            for h in range(1, 8):
                emit_zproj_chunk(0, h)
            emit_vproj(0, 0, 8)

            # ---- steady state ----
            for b in range(BPC):
                for h in range(8):
                    if b == 0 and h == 0:
                        continue  # emitted in prologue
                    prev = pend
                    pend = emit_head(b, h, pend)
                    if prev is not None and prev[1] == 7:
                        emit_norm(prev[0])
                    if b + 1 < BPC:
                        if h == 2:
                            ctx[b + 1] = {"x": prep_x(b + 1)}
                            prep_xodd(b + 1)
                        elif h in (4, 5, 6, 7):
                            for hc in (2 * (h - 4), 2 * (h - 4) + 1):
                                emit_zproj_chunk(b + 1, hc)
                        if h == 6:
                            emit_vproj(b + 1, 0, 4)
                        elif h == 7:
                            emit_vproj(b + 1, 4, 8)

            # drain: final AV for the last head, then its batch's norm
            bl, hl, ptts = pend
            ops_a = pso.tile([65, 512], F32, name="ps_oa_last", tag="pso")
            ops_b = pso.tile([65, 512], F32, name="ps_ob_last", tag="pso")
            st = (bl, hl, ptts, ops_a, ops_b)
            for j in range(8):
                emit_av_chunk(st, j)
            emit_av_evac(st)
            emit_norm(bl)

    nc.compile()
    return nc


def _prep_consts(Wq, bq, Wk, bk, Wv, bv, rel_h, rel_w):
    # interleaved Z weights: chunk h rows 0-63 = Wk head h, rows 64-127 = Wq
    Wz = np.zeros((1024, 512), np.float32)
    bzv = np.zeros((1024,), np.float32)
    for h in range(HEAD):
        Wz[h * 128:h * 128 + 64] = Wk[h * 64:(h + 1) * 64]
        Wz[h * 128 + 64:h * 128 + 128] = Wq[h * 64:(h + 1) * 64]
        bzv[h * 128:h * 128 + 64] = bk[h * 64:(h + 1) * 64]
        bzv[h * 128 + 64:h * 128 + 128] = bq[h * 64:(h + 1) * 64]
    wzt = np.ascontiguousarray(Wz.T).reshape(4, 128, 1024).astype(np.float16)
    bz = np.ascontiguousarray(bzv.reshape(8, 128).T).astype(np.float32)

    wvpt = np.zeros((512, 520), np.float32)
    bvp = np.zeros((1, 520), np.float32)
    for h in range(HEAD):
        wvpt[:, h * 65:h * 65 + 64] = Wv[h * 64:(h + 1) * 64, :].T
        bvp[0, h * 65:h * 65 + 64] = bv[h * 64:(h + 1) * 64]
        bvp[0, h * 65 + 64] = 1.0

    mask = np.zeros((8, 512), np.float32)
    for h in range(HEAD):
        mask[h, h * 64:(h + 1) * 64] = 1.0

    pos = (rel_h + rel_w).reshape(HEAD, D, N).astype(np.float16)
    return {
        "wzt": wzt,
        "bz": bz,
        "wvpt": wvpt.reshape(4, 128, 520).astype(np.float16),
        "bvp": bvp.astype(np.float16),
        "mask": mask.astype(ml_dtypes.bfloat16),
        "pos": pos,
    }


_CACHE = {}


def build_in_maps(x, Wq, bq, Wk, bk, Wv, bv, rel_h, rel_w):
    x = np.asarray(x, np.float32)
    consts = _prep_consts(
        *[np.asarray(a, np.float32) for a in (Wq, bq, Wk, bk, Wv, bv, rel_h, rel_w)]
    )
    xh = x.reshape(B, C, N).astype(np.float16)
    in_maps = []
    for c in range(N_CORES):
        m = dict(consts)
        m["xh"] = np.ascontiguousarray(xh[c * BPC:(c + 1) * BPC])
        in_maps.append(m)
    return in_maps


def kernel(x, Wq, bq, Wk, bk, Wv, bv, rel_h, rel_w, reg_qk, reg_v):
    # reg_qk / reg_v are computed-then-dropped by the reference -> unused.
    in_maps = build_in_maps(x, Wq, bq, Wk, bk, Wv, bv, rel_h, rel_w)

    if "nc" not in _CACHE:
        _CACHE["nc"] = build_bass()
    res = run_bass_kernel_spmd(_CACHE["nc"], in_maps, list(range(N_CORES)))
    outs = [np.asarray(r["out"]).astype(np.float32) for r in res.results]
    return np.concatenate(outs, axis=0).reshape(B, C, WD, HD)


if __name__ == "__main__":
    nc = build_bass()
    print("built ok")
